# revision 3
# baseline (speedup 1.0000x reference)
"""Trainium2 Bass kernel for the sparse-conv network (nn_ExampleNet), v4.

Pair-stacked layout: every activation tensor stores ROW PAIRS across the
128 SBUF partitions (64 channels x 2 consecutive rows), and every matmul
computes TWO output rows at once via a [128(K), 128(M)] stationary whose
column halves hold the weights for out-row r / r+1. Moving-data per output
is halved vs the classic 64-wide scheme and no shifted-copy tiles are
needed. fp16 datapath, fp32 PSUM, 32-row output chunks, 8-way SPMD.
"""
from contextlib import ExitStack

import numpy as np
import ml_dtypes

import concourse.bacc as bacc
import concourse.mybir as mybir
import concourse.tile as tile
from concourse.bass_utils import run_bass_kernel_spmd

F32 = mybir.dt.float32
F16 = mybir.dt.float16
F8 = mybir.dt.float8e4
RELU = mybir.ActivationFunctionType.Relu

B, H, W = 4, 256, 256
WP = W + 2
CH = 32             # output rows per chunk
NCH = 8
S = 16              # input-row step per chunk
PITCH4 = 516
RXS = 138           # x slab rows (69 pair-groups)
RM1 = 136           # mask1 slab rows (68 groups)
RM4 = 260           # mask4 slab rows (130 groups)
G1, G2, G3, G4 = 11, 10, 9, 17   # pair-groups per chunk (h1,h2,h3,h4)

_CACHE = {}


def _host_prep(features, coors, w1, b1, w2, b2, w3, b3, wt, bt, w5, b5):
    f16 = np.float16
    fp8 = ml_dtypes.float8_e4m3fn
    bi, yi, xi = coors[:, 0], coors[:, 1], coors[:, 2]
    flat = (bi.astype(np.int64) * H + yi) * W + xi
    dense = np.zeros((B * H * W, 32), np.float32)
    for c in range(32):
        dense[:, c] = np.bincount(flat, weights=features[:, c],
                                  minlength=B * H * W)
    dense = dense.reshape(B, H, W, 32)
    occ = np.bincount(flat, minlength=B * H * W).reshape(B, H, W) > 0
    m0p = np.zeros((B, H + 2, W + 2), bool)
    m0p[:, 1:-1, 1:-1] = occ
    m1 = np.zeros((B, H, W), bool)
    for dy in range(3):
        for dx in range(3):
            m1 |= m0p[:, dy:dy + H, dx:dx + W]
    m4 = np.zeros((B, 2 * H + 1, 2 * W + 1), bool)
    for dy in range(3):
        for dx in range(3):
            m4[:, dy:dy + 2 * H - 1:2, dx:dx + 2 * W - 1:2] |= m1

    wt_eff = wt[::-1, ::-1]
    z32 = np.zeros((32, 64), np.float32)
    z64 = np.zeros((64, 64), np.float32)

    # conv1: 4 matmuls, slots (pr,pc) at partitions 32*(2pr+pc);
    # mm (g_off,b): source row k=2*g_off+pr, col d=b+pc
    wc1 = np.zeros((128, 4, 128), np.float32)
    for i, (g_off, bb) in enumerate([(0, 0), (1, 0), (0, 2), (1, 2)]):
        for pr in (0, 1):
            for pc in (0, 1):
                k, d = 2 * g_off + pr, bb + pc
                sl = slice(32 * (2 * pr + pc), 32 * (2 * pr + pc) + 32)
                if k <= 2 and d <= 2:
                    wc1[sl, i, 0:64] = w1[k, d]
                if 0 <= k - 1 <= 2 and d <= 2:
                    wc1[sl, i, 64:128] = w1[k - 1, d]

    def mk6p(w):
        # 6 matmuls: (g_off,d) for g_off in (0,1), d in (0,1,2); slot pr at
        # partitions 64*pr; A-cols out row r, B-cols out row r+1
        out = np.zeros((128, 6, 128), np.float32)
        for j in range(6):
            g_off, d = j // 3, j % 3
            for pr in (0, 1):
                k = 2 * g_off + pr
                sl = slice(64 * pr, 64 * pr + 64)
                if k <= 2:
                    out[sl, j, 0:64] = w[k, d]
                if 0 <= k - 1 <= 2:
                    out[sl, j, 64:128] = w[k - 1, d]
        return out

    wc2, wc3, wc5 = mk6p(w2), mk6p(w3), mk6p(w5)

    # convT stationaries [128, 9, 128]:
    # 0,1: E[d] (even-la evens), 2,3: E_r2[d], 4,5: E_r01[d] (odd-la),
    # 6: O, 7: O_r2, 8: O_r01
    wct = np.zeros((128, 9, 128), np.float32)
    for d in (0, 1):
        wct[0:64, d, 0:64] = wt_eff[2, 2 - 2 * d]
        wct[64:128, d, 0:64] = wt_eff[0, 2 - 2 * d]
        wct[64:128, d, 64:128] = wt_eff[1, 2 - 2 * d]
        wct[64:128, 2 + d, 0:64] = wt_eff[2, 2 - 2 * d]
        wct[0:64, 4 + d, 0:64] = wt_eff[0, 2 - 2 * d]
        wct[0:64, 4 + d, 64:128] = wt_eff[1, 2 - 2 * d]
    wct[0:64, 6, 0:64] = wt_eff[2, 1]
    wct[64:128, 6, 0:64] = wt_eff[0, 1]
    wct[64:128, 6, 64:128] = wt_eff[1, 1]
    wct[64:128, 7, 0:64] = wt_eff[2, 1]
    wct[0:64, 8, 0:64] = wt_eff[0, 1]
    wct[0:64, 8, 64:128] = wt_eff[1, 1]

    biases = np.zeros((128, 5), np.float32)
    for l, bv in enumerate([b1, b2, b3, bt, b5]):
        biases[0:64, l] = bv
        biases[64:128, l] = bv

    in_maps = []
    for core in range(8):
        b, half = core // 2, core % 2
        A0 = 0 if half == 0 else 128
        U0 = 2 * A0
        xs = np.zeros((32, RXS, WP), np.float32)
        lo, hi = max(0, A0 - 4), min(H, A0 - 4 + RXS)
        xs[:, lo - (A0 - 4):hi - (A0 - 4), 1:W + 1] = \
            dense[b, lo:hi].transpose(2, 0, 1)
        m1s = np.zeros((RM1 + 2, WP), np.float32)
        lo, hi = max(0, A0 - 3), min(H, A0 - 3 + RM1)
        m1s[lo - (A0 - 3):hi - (A0 - 3), 1:W + 1] = m1[b, lo:hi]
        m4s = np.zeros((RM4, PITCH4), np.float32)
        lo, hi = max(0, U0), min(2 * H + 1, U0 + RM4)
        m4s[lo - U0:hi - U0, :2 * W + 1] = m4[b, lo:hi]

        xsf = xs.reshape(32, -1)
        xsf_pad = np.zeros((32, (RXS + 2) * WP), np.float32)
        xsf_pad[:, :RXS * WP] = xsf
        xq = np.zeros((128, RXS // 2, WP), f16)
        for pr in (0, 1):
            for pc in (0, 1):
                gsl = slice(32 * (2 * pr + pc), 32 * (2 * pr + pc) + 32)
                sh = xsf_pad[:, pr * WP + pc:pr * WP + pc + RXS * WP]
                xq[gsl] = sh.reshape(32, RXS // 2, 2 * WP)[:, :, :WP].astype(f16)

        mse = np.zeros((128, RM1 // 2, WP), f16)
        mso = np.zeros((128, RM1 // 2, WP), f16)
        m4p = np.zeros((128, RM4 // 2, PITCH4), fp8)
        for pr in (0, 1):
            sl = slice(64 * pr, 64 * pr + 64)
            mse[sl] = np.broadcast_to(
                m1s[pr:pr + RM1:2][None], (64, RM1 // 2, WP)).astype(f16)
            mso[sl] = np.broadcast_to(
                m1s[1 + pr:1 + pr + RM1:2][None],
                (64, RM1 // 2, WP)).astype(f16)
            m4p[sl] = np.broadcast_to(
                m4s[pr:pr + RM4:2][None], (64, RM4 // 2, PITCH4)).astype(fp8)

        in_maps.append(dict(
            xq=np.ascontiguousarray(xq),
            mse=np.ascontiguousarray(mse), mso=np.ascontiguousarray(mso),
            m4p=np.ascontiguousarray(m4p),
            wc1=wc1.astype(f16), wc2=wc2.astype(f16), wc3=wc3.astype(f16),
            wct=wct.astype(f16), wc5=wc5.astype(f16), biases=biases,
        ))
    return in_maps


def _build_program():
    nc = bacc.Bacc("TRN2", target_bir_lowering=False, debug=False,
                   enable_asserts=True, num_devices=8)

    xq_d = nc.dram_tensor("xq", [128, RXS // 2, WP], F16,
                          kind="ExternalInput").ap()
    mse_d = nc.dram_tensor("mse", [128, RM1 // 2, WP], F16,
                           kind="ExternalInput").ap()
    mso_d = nc.dram_tensor("mso", [128, RM1 // 2, WP], F16,
                           kind="ExternalInput").ap()
    m4_d = nc.dram_tensor("m4p", [128, RM4 // 2, PITCH4], F8,
                          kind="ExternalInput").ap()
    wc1_d = nc.dram_tensor("wc1", [128, 4, 128], F16,
                           kind="ExternalInput").ap()
    wc2_d = nc.dram_tensor("wc2", [128, 6, 128], F16,
                           kind="ExternalInput").ap()
    wc3_d = nc.dram_tensor("wc3", [128, 6, 128], F16,
                           kind="ExternalInput").ap()
    wct_d = nc.dram_tensor("wct", [128, 9, 128], F16,
                           kind="ExternalInput").ap()
    wc5_d = nc.dram_tensor("wc5", [128, 6, 128], F16,
                           kind="ExternalInput").ap()
    bias_d = nc.dram_tensor("biases", [128, 5], F32,
                            kind="ExternalInput").ap()
    out_d = nc.dram_tensor("out", [64, 256 * 511], F16,
                           kind="ExternalOutput").ap()

    with tile.TileContext(nc) as tc, ExitStack() as ctx:
        wp = ctx.enter_context(tc.tile_pool(name="wp", bufs=1))
        xp = ctx.enter_context(tc.tile_pool(name="xp", bufs=2))
        mp = ctx.enter_context(tc.tile_pool(name="mp", bufs=2))
        hp = ctx.enter_context(tc.tile_pool(name="hp", bufs=1))
        pp = ctx.enter_context(tc.tile_pool(name="pp", bufs=2, space="PSUM"))
        op = ctx.enter_context(tc.tile_pool(name="op", bufs=4))

        w1t = wp.tile([128, 4, 128], F16, name="w1t")
        w2t = wp.tile([128, 6, 128], F16, name="w2t")
        w3t = wp.tile([128, 6, 128], F16, name="w3t")
        wtt = wp.tile([128, 9, 128], F16, name="wtt")
        w5t = wp.tile([128, 6, 128], F16, name="w5t")
        bt = wp.tile([128, 5], F32, name="bt")
        nc.sync.dma_start(w1t[:], wc1_d[:])
        nc.sync.dma_start(bt[:], bias_d[:])
        nc.scalar.dma_start(w2t[:], wc2_d[:])
        nc.scalar.dma_start(w3t[:], wc3_d[:])
        nc.gpsimd.dma_start(wtt[:], wct_d[:])
        nc.gpsimd.dma_start(w5t[:], wc5_d[:])

        def conv_pair(inp, wt_, mms, npairs, bias_ap, m_ch, mg0, h_out):
            # out-pairs two at a time: psum [128, np2, 256]
            for p0 in range(0, npairs, 2):
                np2 = min(2, npairs - p0)
                pc = pp.tile([128, 2, 256], F32, name="pc", tag="pc", bufs=3)
                for i, (g_off, d) in enumerate(mms):
                    nc.tensor.matmul(
                        pc[:, 0:np2], wt_[:, i, :],
                        inp[:, p0 + g_off:p0 + g_off + np2, d:d + 256],
                        start=(i == 0), stop=(i == len(mms) - 1))
                dst = h_out[:, p0:p0 + np2, 1:257]
                nc.scalar.activation(dst, pc[:, 0:np2], RELU, bias=bias_ap)
                nc.vector.tensor_mul(
                    dst, dst, m_ch[:, mg0 + p0:mg0 + p0 + np2, 1:257])

        MM1 = [(0, 0), (1, 0), (0, 2), (1, 2)]
        MM6 = [(g, d) for g in (0, 1) for d in (0, 1, 2)]

        def load_chunk(c):
            x_ch = xp.tile([128, 12, WP], F16, name="x_ch", tag="x")
            nc.sync.dma_start(x_ch[:], xq_d[:, 8 * c:8 * c + 12, :])
            me_ch = mp.tile([128, G1, WP], F16, name="me_ch", tag="me")
            nc.sync.dma_start(me_ch[:], mse_d[:, 8 * c:8 * c + G1, :])
            mo_ch = mp.tile([128, G2, WP], F16, name="mo_ch", tag="mo")
            nc.sync.dma_start(mo_ch[:], mso_d[:, 8 * c:8 * c + G2, :])
            m4_ch = mp.tile([128, G4, PITCH4], F8, name="m4_ch", tag="m4")
            nc.scalar.dma_start(m4_ch[:], m4_d[:, 16 * c:16 * c + G4, :])
            return x_ch, me_ch, mo_ch, m4_ch

        def emit_convT(h3, h4, m4_ch):
            for la in range(2 * G3 - 1):   # 0..16
                pe = pp.tile([128, 257], F32, name="pe", tag="pT", bufs=3)
                if la % 2 == 0:
                    g = la // 2
                    nc.tensor.matmul(pe[:], wtt[:, 0, :], h3[:, g, 0:257],
                                     start=True, stop=False)
                    nc.tensor.matmul(pe[:], wtt[:, 1, :], h3[:, g, 1:258],
                                     start=False, stop=True)
                else:
                    g1, g2 = (la - 1) // 2, (la + 1) // 2
                    nc.tensor.matmul(pe[:], wtt[:, 2, :], h3[:, g1, 0:257],
                                     start=True, stop=False)
                    nc.tensor.matmul(pe[:], wtt[:, 3, :], h3[:, g1, 1:258],
                                     start=False, stop=False)
                    nc.tensor.matmul(pe[:], wtt[:, 4, :], h3[:, g2, 0:257],
                                     start=False, stop=False)
                    nc.tensor.matmul(pe[:], wtt[:, 5, :], h3[:, g2, 1:258],
                                     start=False, stop=True)
                de = h4[:, la, 0:513:2]
                nc.scalar.activation(de, pe[:], RELU, bias=bt[:, 3:4])
                nc.vector.tensor_mul(de, de, m4_ch[:, la, 0:513:2])

                po = pp.tile([128, 256], F32, name="po", tag="pT", bufs=3)
                if la % 2 == 0:
                    g = la // 2
                    nc.tensor.matmul(po[:], wtt[:, 6, :], h3[:, g, 1:257],
                                     start=True, stop=True)
                else:
                    g1, g2 = (la - 1) // 2, (la + 1) // 2
                    nc.tensor.matmul(po[:], wtt[:, 7, :], h3[:, g1, 1:257],
                                     start=True, stop=False)
                    nc.tensor.matmul(po[:], wtt[:, 8, :], h3[:, g2, 1:257],
                                     start=False, stop=True)
                do = h4[:, la, 1:513:2]
                nc.scalar.activation(do, po[:], RELU, bias=bt[:, 3:4])
                nc.vector.tensor_mul(do, do, m4_ch[:, la, 1:513:2])

        def emit_conv5(h4, c, ms):
            for m in ms:
                p5 = pp.tile([128, 512], F32, name="p5", tag="p5")
                for j, (g_off, d) in enumerate(MM6):
                    nc.tensor.matmul(p5[:], w5t[:, j, :],
                                     h4[:, m + g_off, d:d + 512],
                                     start=(j == 0), stop=(j == 5))
                out_sb = op.tile([128, 511], F16, name="out_sb", tag="o",
                                 bufs=6)
                nc.scalar.activation(out_sb[:], p5[:, 0:511], RELU,
                                     bias=bt[:, 4:5])
                r = CH * c + 2 * m
                nc.gpsimd.dma_start(out_d[:, r * 511:(r + 1) * 511],
                                    out_sb[0:64])
                nc.gpsimd.dma_start(out_d[:, (r + 1) * 511:(r + 2) * 511],
                                    out_sb[64:128])

        prev = None
        loads = None
        for it in range(NCH + 1):
            if it == 0:
                loads = load_chunk(0)
            if prev is not None:
                h3p, h4p, m4p, cp = prev
                emit_convT(h3p, h4p, m4p)
            if it < NCH:
                x_ch, me_ch, mo_ch, m4_ch = loads
                h1 = hp.tile([128, G1, WP], F16, name="h1", tag="h1")
                h2 = hp.tile([128, G2, WP], F16, name="h2", tag="h2")
                h3 = hp.tile([128, G3, WP], F16, name="h3", tag="h3", bufs=2)
                for h_ in (h1, h2, h3):
                    nc.gpsimd.memset(h_[:, :, 0:1], 0)
                    nc.gpsimd.memset(h_[:, :, 257:258], 0)
                conv_pair(x_ch, w1t, MM1, G1, bt[:, 0:1], me_ch, 0, h1)
                if it + 1 < NCH:
                    loads = load_chunk(it + 1)
                conv_pair(h1, w2t, MM6, G2, bt[:, 1:2], mo_ch, 0, h2)
            if prev is not None:
                emit_conv5(h4p, cp, range(0, 8))
            if it < NCH:
                conv_pair(h2, w3t, MM6, G3, bt[:, 2:3], me_ch, 1, h3)
            if prev is not None:
                emit_conv5(h4p, cp, range(8, 16))
            if it < NCH:
                h4 = hp.tile([128, G4, PITCH4], F16, name="h4", tag="h4")
                nc.gpsimd.memset(h4[:, :, 513:516], 0)
                prev = (h3, h4, m4_ch, it)
            else:
                prev = None

    nc.compile()
    return nc


def kernel(**inputs):
    features = np.asarray(inputs["features"], np.float32)
    coors = np.asarray(inputs["coors"], np.int32)
    args = [np.asarray(inputs[k], np.float32) for k in
            ("w1", "b1", "w2", "b2", "w3", "b3", "wt", "bt", "w5", "b5")]
    in_maps = _host_prep(features, coors, *args)
    if "nc" not in _CACHE:
        _CACHE["nc"] = _build_program()
    res = run_bass_kernel_spmd(_CACHE["nc"], in_maps,
                               core_ids=list(range(8)), trace=False)
    full = np.zeros((B, 511, 511, 64), np.float32)
    for core in range(8):
        o = np.asarray(res.results[core]["out"]).astype(np.float32)
        o = o.reshape(64, 256, 511)
        b, half = core // 2, core % 2
        if half == 0:
            full[b, 0:256] = o.transpose(1, 2, 0)
        else:
            full[b, 256:511] = o[:, 0:255].transpose(1, 2, 0)
    return full
